# revision 27
# baseline (speedup 1.0000x reference)
"""Trainium2 Bass kernel for an AttentionBlock (GroupNorm + MHA + proj + residual).

Shapes (hardcoded): x (16, 512, 32, 32) f32, 8 heads (ch=64), GN groups=32,
w_qkv (1536, 512), w_proj (512, 512).

Strategy: data-parallel over batch across 8 NeuronCores (2 batches/core, no
collectives). All large matmuls run in float32r (full PE rate for free dim
>= 256). Scores are computed transposed (s on partitions, t free) so the
softmax denominator falls out of the attn@v matmul as a 65th output row
(ones column appended to v^T); no cross-partition reductions anywhere.
GroupNorm statistics use DVE reductions + a tiny block-diagonal matmul, and
rsqrt is computed with a DVE-only Newton iteration (no ACT table swaps —
ScalarE does nothing but exp, which is the bottleneck engine).

Software pipelining across batches (the ACT/exp stream must not starve):
batch b+1's x DMA (Pool queue) and GroupNorm statistics (DVE-only, cannot
stall the PE) are issued before heads(b) so they execute under it; the rest
of b+1's front-end (GN finish + qkv + v^T) is emitted after heads(b) but
BEFORE proj(b), so the PE covers proj's wait on the last softmax-normalize
with independent work and enters heads(b+1) without a dry ACT queue.
"""
import numpy as np
import ml_dtypes
from contextlib import ExitStack

import concourse.bass as bass
import concourse.mybir as mybir
import concourse.tile as tile
from concourse import bacc
from concourse.bass_utils import run_bass_kernel_spmd

F32 = mybir.dt.float32
F32R = mybir.dt.float32r
BF16 = mybir.dt.bfloat16
AF = mybir.ActivationFunctionType
OP = mybir.AluOpType

B, C, H, W = 16, 512, 32, 32
N = H * W            # 1024
NHEADS = 8
CH = C // NHEADS     # 64
NGROUPS = 32
GSIZE = C // NGROUPS  # 16 channels per group
EPS = 1e-5
NCORES = 8
BPC = B // NCORES    # batches per core = 2
NT = C // 128        # channel tiles per batch = 4
VW = NHEADS * (CH + 1)  # v_ext free width = 520

_cached = {}


def _build(dbg=False, reps=1, num_devices=NCORES):
    nc = bacc.Bacc("TRN2", target_bir_lowering=False, debug=False,
                   num_devices=num_devices)

    xd = nc.dram_tensor("x", [BPC, C, N], F32, kind="ExternalInput").ap()
    wqk_d = nc.dram_tensor("wqk_t", [C, 2 * C], BF16, kind="ExternalInput").ap()
    wv_d = nc.dram_tensor("wv_ext", [C, VW], BF16, kind="ExternalInput").ap()
    wp_d = nc.dram_tensor("wp_t", [C, C], BF16, kind="ExternalInput").ap()
    bqk_d = nc.dram_tensor("bqk", [128, 8], F32, kind="ExternalInput").ap()
    bv_d = nc.dram_tensor("bv_bc", [128, VW], F32, kind="ExternalInput").ap()
    bp_d = nc.dram_tensor("bp", [128, NT], F32, kind="ExternalInput").ap()
    gam_d = nc.dram_tensor("gamma_t", [128, NT], F32, kind="ExternalInput").ap()
    bet_d = nc.dram_tensor("beta_t", [128, NT], F32, kind="ExternalInput").ap()
    bd_d = nc.dram_tensor("blockdiag16", [128, 8], F32, kind="ExternalInput").ap()
    bc_d = nc.dram_tensor("bcast16", [8, 128], F32, kind="ExternalInput").ap()
    outd = nc.dram_tensor("out", [BPC, C, N], F32, kind="ExternalOutput").ap()

    with tile.TileContext(nc) as tc, ExitStack() as ctx:
        wpool = ctx.enter_context(tc.tile_pool(name="weights", bufs=1))
        xpool = ctx.enter_context(tc.tile_pool(name="x", bufs=2))
        xnpool = ctx.enter_context(tc.tile_pool(name="xn", bufs=2))
        qkpool = ctx.enter_context(tc.tile_pool(name="qk", bufs=2))
        vpool = ctx.enter_context(tc.tile_pool(name="v", bufs=2))
        hpool = ctx.enter_context(tc.tile_pool(name="h", bufs=2))
        ppool = ctx.enter_context(tc.tile_pool(name="p", bufs=8))
        opool = ctx.enter_context(tc.tile_pool(name="o", bufs=4))
        small = ctx.enter_context(tc.tile_pool(name="small", bufs=2))
        scr = ctx.enter_context(tc.tile_pool(name="scr", bufs=1))
        ps_sc = ctx.enter_context(tc.tile_pool(name="ps_sc", bufs=2, space="PSUM"))
        ps_h = ctx.enter_context(tc.tile_pool(name="ps_h", bufs=2, space="PSUM"))

        wqk_r, wv_r, wp_r = [], [], []
        for k in range(NT):
            wr = wpool.tile([128, 2 * C], BF16, tag=f"wqk{k}")
            nc.sync.dma_start(wr[:], wqk_d[128 * k:128 * (k + 1), :])
            wqk_r.append(wr)
        for k in range(NT):
            wr = wpool.tile([128, VW], BF16, tag=f"wv{k}")
            nc.sync.dma_start(wr[:], wv_d[128 * k:128 * (k + 1), :])
            wv_r.append(wr)
        for k in range(NT):
            wr = wpool.tile([128, C], BF16, tag=f"wp{k}")
            nc.sync.dma_start(wr[:], wp_d[128 * k:128 * (k + 1), :])
            wp_r.append(wr)

        bqk = wpool.tile([128, 8], F32, tag="bqk")
        nc.sync.dma_start(bqk[:], bqk_d[:])
        bv = wpool.tile([128, VW], F32, tag="bv")
        nc.sync.dma_start(bv[:], bv_d[:])
        bp = wpool.tile([128, NT], F32, tag="bp")
        nc.sync.dma_start(bp[:], bp_d[:])
        gam = wpool.tile([128, NT], F32, tag="gam")
        nc.sync.dma_start(gam[:], gam_d[:])
        bet = wpool.tile([128, NT], F32, tag="bet")
        nc.sync.dma_start(bet[:], bet_d[:])
        bd16 = wpool.tile([128, 8], F32, tag="bd16")
        nc.sync.dma_start(bd16[:], bd_d[:])
        bc16 = wpool.tile([8, 128], F32, tag="bc16")
        nc.sync.dma_start(bc16[:], bc_d[:])

        class St:
            pass

        def emit_load(st):
            # Pool's DGE queue: issued before the previous batch's heads, so
            # the transfer lands while ACT is busy with exp
            st.x_sb = xpool.tile([128, NT * N], F32, tag="x", name="x_sb")
            for j in range(NT):
                nc.gpsimd.dma_start(st.x_sb[:, N * j:N * (j + 1)],
                                    xd[st.b, 128 * j:128 * (j + 1), :])

        def emit_stats(st):
            # DVE-only: runs under the previous batch's heads
            st.stat = small.tile([128, 8], F32, tag="stat", name="stat")
            sq = scr.tile([128, N], F32, tag="sq")
            for j in range(NT):
                nc.vector.reduce_sum(st.stat[:, j:j + 1],
                                     st.x_sb[:, N * j:N * (j + 1)],
                                     axis=mybir.AxisListType.X)
                nc.vector.scalar_tensor_tensor(
                    sq[:], st.x_sb[:, N * j:N * (j + 1)], 1.0,
                    st.x_sb[:, N * j:N * (j + 1)],
                    op0=OP.bypass, op1=OP.mult,
                    accum_out=st.stat[:, 4 + j:5 + j])

        def emit_gn_rest(st):
            ps_st = ps_sc.tile([8, 8], F32, tag="sc")
            nc.tensor.matmul(ps_st[:], bd16[:], st.stat[:], start=True, stop=True)
            inv = 1.0 / (GSIZE * N)
            mean8 = small.tile([8, 8], F32, tag="mean8")
            nc.vector.tensor_scalar_mul(mean8[:, 0:4], ps_st[:, 0:4], inv)
            ex2 = small.tile([8, 4], F32, tag="ex2")
            nc.vector.tensor_scalar_mul(ex2[:], ps_st[:, 4:8], inv)
            m2 = small.tile([8, 4], F32, tag="m2")
            nc.vector.tensor_mul(m2[:], mean8[:, 0:4], mean8[:, 0:4])
            veps = small.tile([8, 4], F32, tag="veps")
            nc.vector.scalar_tensor_tensor(veps[:], ex2[:], EPS, m2[:],
                                           op0=OP.add, op1=OP.subtract)
            r_cur = small.tile([8, 4], F32, tag="r0")
            nc.vector.tensor_scalar(r_cur[:], veps[:], -0.5, 1.5,
                                    op0=OP.mult, op1=OP.add)
            for it in range(3):
                t1 = small.tile([8, 4], F32, tag=f"nt1_{it}")
                nc.vector.tensor_mul(t1[:], r_cur[:], r_cur[:])
                t2 = small.tile([8, 4], F32, tag=f"nt2_{it}")
                nc.vector.scalar_tensor_tensor(t2[:], t1[:], -0.5, veps[:],
                                               op0=OP.mult, op1=OP.mult)
                t3 = small.tile([8, 4], F32, tag=f"nt3_{it}")
                nc.vector.tensor_scalar_add(t3[:], t2[:], 1.5)
                r_nxt = small.tile([8, 4], F32, tag=f"nr_{it}")
                nc.vector.tensor_mul(r_nxt[:], r_cur[:], t3[:])
                r_cur = r_nxt
            nc.vector.tensor_copy(mean8[:, 4:8], r_cur[:])
            ps_bc = ps_sc.tile([128, 8], F32, tag="sc")
            nc.tensor.matmul(ps_bc[:], bc16[:], mean8[:], start=True, stop=True)
            A_ch = small.tile([128, NT], F32, tag="A_ch")
            nc.vector.tensor_mul(A_ch[:], gam[:], ps_bc[:, 4:8])
            tB = small.tile([128, NT], F32, tag="tB")
            nc.vector.tensor_mul(tB[:], ps_bc[:, 0:4], A_ch[:])
            B_ch = small.tile([128, NT], F32, tag="B_ch")
            nc.vector.scalar_tensor_tensor(B_ch[:], tB[:], -1.0, bet[:],
                                           op0=OP.mult, op1=OP.add)
            st.xn = xnpool.tile([128, NT * N], BF16, tag="xn", name="xn")
            for j in range(NT):
                nc.vector.tensor_scalar(st.xn[:, N * j:N * (j + 1)],
                                        st.x_sb[:, N * j:N * (j + 1)],
                                        A_ch[:, j:j + 1], B_ch[:, j:j + 1],
                                        op0=OP.mult, op1=OP.add)

        def emit_qkv(st):
            # qk layout: cols 0..4095 = q (4 ch-tiles), 4096..8191 = k
            st.qk = qkpool.tile([128, 8 * N], BF16, tag="qk", name="qk")
            for o in range(8):
                for nh in range(2):
                    pq = ps_h.tile([128, 512], F32, tag="hacc",
                                   name=f"pq{o}_{nh}")
                    for k in range(NT):
                        nc.tensor.matmul(
                            pq[:],
                            wqk_r[k][:, 128 * o:128 * (o + 1)],
                            st.xn[:, N * k + 512 * nh:N * k + 512 * (nh + 1)],
                            start=(k == 0), stop=(k == NT - 1))
                    nc.vector.tensor_scalar_add(
                        st.qk[:, N * o + 512 * nh:N * o + 512 * (nh + 1)],
                        pq[:], bqk[:, o:o + 1])

        def emit_v(st):
            st.vv = vpool.tile([128, 8 * VW], BF16, tag="vv", name="vv")
            for ntile in range(8):
                for chh in range(2):
                    pv = ps_h.tile([128, 260], F32, tag="hacc",
                                   name=f"pv{ntile}_{chh}")
                    for k in range(NT):
                        nc.tensor.matmul(
                            pv[:],
                            st.xn[:, N * k + 128 * ntile:N * k + 128 * (ntile + 1)],
                            wv_r[k][:, 260 * chh:260 * (chh + 1)],
                            start=(k == 0), stop=(k == NT - 1))
                    nc.vector.tensor_add(
                        st.vv[:, VW * ntile + 260 * chh:
                              VW * ntile + 260 * (chh + 1)],
                        pv[:], bv[:, 260 * chh:260 * (chh + 1)])

        def emit_heads(st):
            qk, vv = st.qk, st.vv
            st.hall = hpool.tile([128, NT * N], BF16, tag="hall", name="hall")
            hall = st.hall

            def make_attnv(phs_, pr_):
                def attnv(side, j, p_tile):
                    head = 2 * pr_ + side
                    for th in range(2):
                        nc.tensor.matmul(
                            phs_[side][:, 512 * th:512 * (th + 1)],
                            vv[:, VW * j + (CH + 1) * head:
                               VW * j + (CH + 1) * head + CH + 1],
                            p_tile[:, 512 * th:512 * (th + 1)],
                            start=(j == 0), stop=(j == 7))
                return attnv

            def emit_tail(tail_):
                attnv_, phs_, p_prev_, pr_ = tail_
                for side in range(2):
                    attnv_(side, 7, p_prev_[side])
                for side in range(2):
                    rec = small.tile([1, N], F32, tag="rec")
                    nc.vector.reciprocal(rec[:], phs_[side][64:65, :])
                    rb = scr.tile([64, N], F32, tag="rb", bufs=2)
                    nc.gpsimd.partition_broadcast(rb[:], rec[:])
                    nc.vector.tensor_mul(
                        hall[64 * side:64 * side + 64, N * pr_:N * (pr_ + 1)],
                        phs_[side][0:64, :], rb[:])

            tail = None
            for pr in range(4):
                q_base = N * pr
                k_base = 4 * N + N * pr
                phs = [ps_h.tile([65, N], F32, tag="hacc", name=f"phA{pr}"),
                       ps_h.tile([65, N], F32, tag="hacc", name=f"phB{pr}")]
                attnv = make_attnv(phs, pr)

                p_prev = [None, None]
                for j in range(8):
                    for side in range(2):
                        pb = 64 * side
                        sc = ps_sc.tile([128, N], F32, tag="sc")
                        for th in range(2):
                            nc.tensor.matmul(
                                sc[:, 512 * th:512 * (th + 1)],
                                qk[pb:pb + 64, k_base + 128 * j:k_base + 128 * (j + 1)],
                                qk[pb:pb + 64, q_base + 512 * th:q_base + 512 * (th + 1)],
                                start=True, stop=True,
                                tile_position=(pb, 0))
                        p_t = ppool.tile([128, N], BF16, tag="p")
                        nc.scalar.activation(p_t[:], sc[:], AF.Exp)
                        if p_prev[side] is not None:
                            attnv(side, j - 1, p_prev[side])
                        p_prev[side] = p_t
                        if j == 0 and side == 1 and tail is not None:
                            emit_tail(tail)
                            tail = None
                tail = (attnv, phs, p_prev, pr)
            emit_tail(tail)

        def emit_proj(st):
            for o in range(NT):
                for nh in range(2):
                    pp = ps_h.tile([128, 512], F32, tag="hacc")
                    for k in range(NT):
                        nc.tensor.matmul(
                            pp[:],
                            wp_r[k][:, 128 * o:128 * (o + 1)],
                            st.hall[:, N * k + 512 * nh:N * k + 512 * (nh + 1)],
                            start=(k == 0), stop=(k == NT - 1))
                    ot = opool.tile([128, 512], F32, tag="ot")
                    nc.vector.scalar_tensor_tensor(
                        ot[:], pp[:], bp[:, o:o + 1],
                        st.x_sb[:, N * o + 512 * nh:N * o + 512 * (nh + 1)],
                        op0=OP.add, op1=OP.add)
                    nc.sync.dma_start(
                        outd[st.b, 128 * o:128 * (o + 1), 512 * nh:512 * (nh + 1)],
                        ot[:])

        bs = [b for _ in range(reps) for b in range(BPC)]
        st = St()
        st.b = bs[0]
        emit_load(st)
        emit_stats(st)
        emit_gn_rest(st)
        emit_qkv(st)
        emit_v(st)
        for idx in range(len(bs)):
            if idx + 1 < len(bs):
                nxt = St()
                nxt.b = bs[idx + 1]
                emit_load(nxt)   # Pool DMA: transfers during heads(st)
                emit_stats(nxt)  # DVE-only: runs under heads(st)
            else:
                nxt = None
            emit_heads(st)
            if nxt is not None:
                # front-end of b+1 before proj(b): the PE covers proj's wait
                # on the last normalize, and heads(b+1) starts with qk ready
                emit_gn_rest(nxt)
                emit_qkv(nxt)
                emit_v(nxt)
            emit_proj(st)
            if nxt is not None:
                st = nxt

    nc.compile()
    return nc


def _prep_shared(w_qkv, b_qkv, w_proj, b_proj, gamma, beta):
    qs = 1.0 / np.sqrt(np.sqrt(float(CH)))  # ch**-0.25
    s2 = qs * qs
    r = np.arange(3 * C).reshape(NHEADS, 3, CH)
    idx_q, idx_k, idx_v = r[:, 0].ravel(), r[:, 1].ravel(), r[:, 2].ravel()
    wqk_t = np.ascontiguousarray(
        np.concatenate([w_qkv[idx_q], w_qkv[idx_k]], axis=0).T).astype(np.float32)
    wqk_t[:, :C] *= s2
    bqk_full = np.concatenate([b_qkv[idx_q], b_qkv[idx_k]])
    bqk_full[:C] *= s2
    bqk = np.ascontiguousarray(bqk_full.reshape(8, 128).T).astype(np.float32)

    wv = w_qkv[idx_v]
    bv_src = b_qkv[idx_v]
    wv_ext = np.zeros((C, VW), np.float32)
    bv_ext = np.zeros((VW,), np.float32)
    for h in range(NHEADS):
        wv_ext[:, (CH + 1) * h:(CH + 1) * h + CH] = wv[CH * h:CH * (h + 1), :].T
        bv_ext[(CH + 1) * h:(CH + 1) * h + CH] = bv_src[CH * h:CH * (h + 1)]
        bv_ext[(CH + 1) * h + CH] = 1.0
    bv_bc = np.ascontiguousarray(np.broadcast_to(bv_ext, (128, VW))).astype(np.float32)

    wp_t = np.ascontiguousarray(w_proj.T).astype(np.float32)
    bp = np.ascontiguousarray(b_proj.reshape(NT, 128).T).astype(np.float32)
    gamma_t = np.ascontiguousarray(gamma.reshape(NT, 128).T).astype(np.float32)
    beta_t = np.ascontiguousarray(beta.reshape(NT, 128).T).astype(np.float32)
    blockdiag16 = np.kron(np.eye(8, dtype=np.float32), np.ones((GSIZE, 1), np.float32))
    bcast16 = np.ascontiguousarray(blockdiag16.T)
    bf = ml_dtypes.bfloat16
    wqk_t, wv_ext, wp_t = (a.astype(bf) for a in (wqk_t, wv_ext, wp_t))
    return dict(wqk_t=wqk_t, bqk=bqk, wv_ext=wv_ext, bv_bc=bv_bc, wp_t=wp_t,
                bp=bp, gamma_t=gamma_t, beta_t=beta_t,
                blockdiag16=blockdiag16, bcast16=bcast16)


def kernel(x, gamma, beta, w_qkv, b_qkv, w_proj, b_proj):
    x = np.asarray(x, dtype=np.float32)
    shared = _prep_shared(np.asarray(w_qkv, np.float32), np.asarray(b_qkv, np.float32),
                          np.asarray(w_proj, np.float32), np.asarray(b_proj, np.float32),
                          np.asarray(gamma, np.float32), np.asarray(beta, np.float32))
    x6 = x.reshape(B, C, N)
    in_maps = [dict(x=np.ascontiguousarray(x6[BPC * i:BPC * (i + 1)]), **shared)
               for i in range(NCORES)]
    if "nc" not in _cached:
        _cached["nc"] = _build()
    res = run_bass_kernel_spmd(_cached["nc"], in_maps, list(range(NCORES)))
    out = np.empty((B, C, N), np.float32)
    for i in range(NCORES):
        out[BPC * i:BPC * (i + 1)] = res.results[i]["out"]
    return out.reshape(B, C, H, W)


# revision 29
# speedup vs baseline: 1.1719x; 1.1719x over previous
"""Trainium2 Bass kernel for an AttentionBlock (GroupNorm + MHA + proj + residual).

Shapes (hardcoded): x (16, 512, 32, 32) f32, 8 heads (ch=64), GN groups=32,
w_qkv (1536, 512), w_proj (512, 512).

Strategy: data-parallel over batch across 8 NeuronCores (2 batches/core, no
collectives). All large matmuls run in float32r (full PE rate for free dim
>= 256). Scores are computed transposed (s on partitions, t free) so the
softmax denominator falls out of the attn@v matmul as a 65th output row
(ones column appended to v^T); no cross-partition reductions anywhere.
GroupNorm statistics use DVE reductions + a tiny block-diagonal matmul, and
rsqrt is computed with a DVE-only Newton iteration (no ACT table swaps —
ScalarE does nothing but exp, which is the bottleneck engine).

Software pipelining across batches (the ACT/exp stream must not starve):
batch b+1's x DMA (Pool queue) and GroupNorm statistics (DVE-only, cannot
stall the PE) are issued before heads(b) so they execute under it; the rest
of b+1's front-end (GN finish + qkv + v^T) is emitted after heads(b) but
BEFORE proj(b), so the PE covers proj's wait on the last softmax-normalize
with independent work and enters heads(b+1) without a dry ACT queue.
"""
import numpy as np
import ml_dtypes
from contextlib import ExitStack

import concourse.bass as bass
import concourse.mybir as mybir
import concourse.tile as tile
from concourse import bacc
from concourse.bass_utils import run_bass_kernel_spmd

F32 = mybir.dt.float32
F32R = mybir.dt.float32r
BF16 = mybir.dt.bfloat16
FP8 = mybir.dt.float8e4
DR = mybir.MatmulPerfMode.DoubleRow
AF = mybir.ActivationFunctionType
OP = mybir.AluOpType

B, C, H, W = 16, 512, 32, 32
N = H * W            # 1024
NHEADS = 8
CH = C // NHEADS     # 64
NGROUPS = 32
GSIZE = C // NGROUPS  # 16 channels per group
EPS = 1e-5
NCORES = 8
BPC = B // NCORES    # batches per core = 2
NT = C // 128        # channel tiles per batch = 4
VW = NHEADS * (CH + 1)  # v_ext free width = 520
SW = 32.0  # fp8 weight prescale
HS = 32.0  # attn-out prescale
CHK = (0, 2, 1, 3)

_cached = {}


def _build(dbg=False, reps=1, num_devices=NCORES):
    nc = bacc.Bacc("TRN2", target_bir_lowering=False, debug=False,
                   num_devices=num_devices)

    xd = nc.dram_tensor("x", [BPC, C, N], F32, kind="ExternalInput").ap()
    wqk_d = nc.dram_tensor("wqk8", [128, 4 * 1024], FP8, kind="ExternalInput").ap()
    wv_d = nc.dram_tensor("wv8", [128, 4 * VW], FP8, kind="ExternalInput").ap()
    wp_d = nc.dram_tensor("wp8", [128, 4 * C], FP8, kind="ExternalInput").ap()
    bqk_d = nc.dram_tensor("bqk", [128, 8], F32, kind="ExternalInput").ap()
    bv_d = nc.dram_tensor("bv_bc", [128, VW], F32, kind="ExternalInput").ap()
    bp_d = nc.dram_tensor("bp", [128, NT], F32, kind="ExternalInput").ap()
    gam_d = nc.dram_tensor("gamma_t", [128, NT], F32, kind="ExternalInput").ap()
    bet_d = nc.dram_tensor("beta_t", [128, NT], F32, kind="ExternalInput").ap()
    bd_d = nc.dram_tensor("blockdiag16", [128, 8], F32, kind="ExternalInput").ap()
    bc_d = nc.dram_tensor("bcast16", [8, 128], F32, kind="ExternalInput").ap()
    outd = nc.dram_tensor("out", [BPC, C, N], F32, kind="ExternalOutput").ap()

    with tile.TileContext(nc) as tc, ExitStack() as ctx:
        wpool = ctx.enter_context(tc.tile_pool(name="weights", bufs=1))
        xpool = ctx.enter_context(tc.tile_pool(name="x", bufs=2))
        xnpool = ctx.enter_context(tc.tile_pool(name="xn", bufs=2))
        xbpool = ctx.enter_context(tc.tile_pool(name="xb", bufs=2))
        qkpool = ctx.enter_context(tc.tile_pool(name="qk", bufs=2))
        vpool = ctx.enter_context(tc.tile_pool(name="v", bufs=2))
        hpool = ctx.enter_context(tc.tile_pool(name="h", bufs=2))
        ppool = ctx.enter_context(tc.tile_pool(name="p", bufs=5))
        opool = ctx.enter_context(tc.tile_pool(name="o", bufs=4))
        small = ctx.enter_context(tc.tile_pool(name="small", bufs=2))
        scr = ctx.enter_context(tc.tile_pool(name="scr", bufs=1))
        ps_sc = ctx.enter_context(tc.tile_pool(name="ps_sc", bufs=2, space="PSUM"))
        ps_h = ctx.enter_context(tc.tile_pool(name="ps_h", bufs=2, space="PSUM"))

        wqk_sb = wpool.tile([128, 4 * 1024], FP8, tag="wqk")
        nc.sync.dma_start(wqk_sb[:], wqk_d[:])
        wv_sb = wpool.tile([128, 4 * VW], FP8, tag="wv")
        nc.sync.dma_start(wv_sb[:], wv_d[:])
        wp_sb = wpool.tile([128, 4 * C], FP8, tag="wp")
        nc.sync.dma_start(wp_sb[:], wp_d[:])
        wqk4 = wqk_sb[:].rearrange("p (j i m) -> p j i m", j=2, i=2)
        wv4 = wv_sb[:].rearrange("p (j i w) -> p j i w", j=2, i=2)
        wp4 = wp_sb[:].rearrange("p (j i m) -> p j i m", j=2, i=2)

        bqk = wpool.tile([128, 8], F32, tag="bqk")
        nc.sync.dma_start(bqk[:], bqk_d[:])
        bv = wpool.tile([128, VW], F32, tag="bv")
        nc.sync.dma_start(bv[:], bv_d[:])
        bp = wpool.tile([128, NT], F32, tag="bp")
        nc.sync.dma_start(bp[:], bp_d[:])
        gam = wpool.tile([128, NT], F32, tag="gam")
        nc.sync.dma_start(gam[:], gam_d[:])
        bet = wpool.tile([128, NT], F32, tag="bet")
        nc.sync.dma_start(bet[:], bet_d[:])
        bd16 = wpool.tile([128, 8], F32, tag="bd16")
        nc.sync.dma_start(bd16[:], bd_d[:])
        bc16 = wpool.tile([8, 128], F32, tag="bc16")
        nc.sync.dma_start(bc16[:], bc_d[:])

        class St:
            pass

        def emit_load(st):
            # Pool's DGE queue: issued before the previous batch's heads, so
            # the transfer lands while ACT is busy with exp
            st.x_sb = xpool.tile([128, NT * N], F32, tag="x", name="x_sb")
            for j in range(NT):
                nc.gpsimd.dma_start(st.x_sb[:, N * j:N * (j + 1)],
                                    xd[st.b, 128 * j:128 * (j + 1), :])

        def emit_stats(st):
            # DVE-only: runs under the previous batch's heads
            st.stat = small.tile([128, 8], F32, tag="stat", name="stat")
            sq = scr.tile([128, N], F32, tag="sq")
            for j in range(NT):
                nc.vector.reduce_sum(st.stat[:, j:j + 1],
                                     st.x_sb[:, N * j:N * (j + 1)],
                                     axis=mybir.AxisListType.X)
                nc.vector.scalar_tensor_tensor(
                    sq[:], st.x_sb[:, N * j:N * (j + 1)], 1.0,
                    st.x_sb[:, N * j:N * (j + 1)],
                    op0=OP.bypass, op1=OP.mult,
                    accum_out=st.stat[:, 4 + j:5 + j])

        def emit_gn_rest(st):
            ps_st = ps_sc.tile([8, 8], F32, tag="sc")
            nc.tensor.matmul(ps_st[:], bd16[:], st.stat[:], start=True, stop=True)
            inv = 1.0 / (GSIZE * N)
            mean8 = small.tile([8, 8], F32, tag="mean8")
            nc.vector.tensor_scalar_mul(mean8[:, 0:4], ps_st[:, 0:4], inv)
            ex2 = small.tile([8, 4], F32, tag="ex2")
            nc.vector.tensor_scalar_mul(ex2[:], ps_st[:, 4:8], inv)
            m2 = small.tile([8, 4], F32, tag="m2")
            nc.vector.tensor_mul(m2[:], mean8[:, 0:4], mean8[:, 0:4])
            veps = small.tile([8, 4], F32, tag="veps")
            nc.vector.scalar_tensor_tensor(veps[:], ex2[:], EPS, m2[:],
                                           op0=OP.add, op1=OP.subtract)
            r_cur = small.tile([8, 4], F32, tag="r0")
            nc.vector.tensor_scalar(r_cur[:], veps[:], -0.5, 1.5,
                                    op0=OP.mult, op1=OP.add)
            for it in range(3):
                t1 = small.tile([8, 4], F32, tag=f"nt1_{it}")
                nc.vector.tensor_mul(t1[:], r_cur[:], r_cur[:])
                t2 = small.tile([8, 4], F32, tag=f"nt2_{it}")
                nc.vector.scalar_tensor_tensor(t2[:], t1[:], -0.5, veps[:],
                                               op0=OP.mult, op1=OP.mult)
                t3 = small.tile([8, 4], F32, tag=f"nt3_{it}")
                nc.vector.tensor_scalar_add(t3[:], t2[:], 1.5)
                r_nxt = small.tile([8, 4], F32, tag=f"nr_{it}")
                nc.vector.tensor_mul(r_nxt[:], r_cur[:], t3[:])
                r_cur = r_nxt
            nc.vector.tensor_copy(mean8[:, 4:8], r_cur[:])
            ps_bc = ps_sc.tile([128, 8], F32, tag="sc")
            nc.tensor.matmul(ps_bc[:], bc16[:], mean8[:], start=True, stop=True)
            A_ch = small.tile([128, NT], F32, tag="A_ch")
            nc.vector.tensor_mul(A_ch[:], gam[:], ps_bc[:, 4:8])
            tB = small.tile([128, NT], F32, tag="tB")
            nc.vector.tensor_mul(tB[:], ps_bc[:, 0:4], A_ch[:])
            B_ch = small.tile([128, NT], F32, tag="B_ch")
            nc.vector.scalar_tensor_tensor(B_ch[:], tB[:], -1.0, bet[:],
                                           op0=OP.mult, op1=OP.add)
            st.xn = xnpool.tile([128, NT * N], FP8, tag="xn", name="xn")
            st.xb = xbpool.tile([128, NT * N], F32, tag="xb", name="xb")
            for j in range(NT):
                nc.vector.tensor_scalar(st.xn[:, N * j:N * (j + 1)],
                                        st.x_sb[:, N * j:N * (j + 1)],
                                        A_ch[:, j:j + 1], B_ch[:, j:j + 1],
                                        op0=OP.mult, op1=OP.add)
                nc.gpsimd.tensor_scalar_add(st.xb[:, N * j:N * (j + 1)],
                                            st.x_sb[:, N * j:N * (j + 1)],
                                            bp[:, j:j + 1])
            st.xn4 = st.xn[:].rearrange("p (j i n) -> p j i n", j=2, i=2)

        def emit_qkv(st):
            # qk layout: cols 0..4095 = q (4 ch-tiles), 4096..8191 = k
            st.qk = qkpool.tile([128, 8 * N], BF16, tag="qk", name="qk")
            for o in range(8):
                pq = ps_h.tile([128, N], F32, tag="hacc", name=f"pq{o}")
                for kp in range(2):
                    for chk in CHK:
                        nc.tensor.matmul(
                            pq[:, 256 * chk:256 * (chk + 1)],
                            wqk4[:, kp, :, 128 * o:128 * (o + 1)],
                            st.xn4[:, kp, :, 256 * chk:256 * (chk + 1)],
                            start=(kp == 0 and chk % 2 == 0),
                            stop=(kp == 1 and chk % 2 == 1), perf_mode=DR)
                ds = (0.125 / SW) if o < 4 else (1.0 / SW)
                nc.vector.tensor_scalar(st.qk[:, N * o:N * (o + 1)],
                                        pq[:], ds, bqk[:, o:o + 1],
                                        op0=OP.mult, op1=OP.add)

        def emit_v(st):
            st.vv = vpool.tile([128, 8 * VW], BF16, tag="vv", name="vv")
            for nb in range(8):
                pv = [ps_h.tile([128, 260], F32, tag="hacc",
                                name=f"pv{nb}_{h}") for h in range(2)]
                for kp in range(2):
                    for cq in range(2):
                        for half in range(2):
                            nc.tensor.matmul(
                                pv[half][:, 130 * cq:130 * (cq + 1)],
                                st.xn4[:, kp, :, 128 * nb:128 * (nb + 1)],
                                wv4[:, kp, :, 260 * half + 130 * cq:
                                    260 * half + 130 * (cq + 1)],
                                start=(kp == 0 and cq == 0),
                                stop=(kp == 1 and cq == 1), perf_mode=DR)
                for half in range(2):
                    nc.vector.scalar_tensor_tensor(
                        st.vv[:, VW * nb + 260 * half:
                              VW * nb + 260 * (half + 1)],
                        pv[half][:], 1.0 / SW, bv[:, 260 * half:260 * (half + 1)],
                        op0=OP.mult, op1=OP.add)

        def emit_heads(st):
            qk, vv = st.qk, st.vv
            st.hall = hpool.tile([128, NT * N], FP8, tag="hall", name="hall")
            hall = st.hall

            def make_attnv(phs_, pr_):
                def attnv(side, j, p_tile):
                    head = 2 * pr_ + side
                    for th in range(2):
                        nc.tensor.matmul(
                            phs_[side][:, 512 * th:512 * (th + 1)],
                            vv[:, VW * j + (CH + 1) * head:
                               VW * j + (CH + 1) * head + CH + 1],
                            p_tile[:, 512 * th:512 * (th + 1)],
                            start=(j == 0), stop=(j == 7))
                return attnv

            def emit_tail(tail_):
                attnv_, phs_, p_prev_, pr_ = tail_
                for side in range(2):
                    attnv_(side, 7, p_prev_[side])
                for side in range(2):
                    hc = scr.tile([65, N], F32, tag="hc", bufs=2)
                    nc.vector.tensor_copy(hc[:], phs_[side][:])
                    rec = small.tile([1, N], F32, tag="rec")
                    nc.vector.reciprocal(rec[:], hc[64:65, :])
                    rb = scr.tile([64, N], F32, tag="rb", bufs=2)
                    nc.gpsimd.partition_broadcast(rb[:], rec[:])
                    nc.vector.scalar_tensor_tensor(
                        hall[64 * side:64 * side + 64, N * pr_:N * (pr_ + 1)],
                        hc[0:64, :], HS, rb[:], op0=OP.mult, op1=OP.mult)

            tail = None
            for pr in range(4):
                q_base = N * pr
                k_base = 4 * N + N * pr
                phs = [ps_h.tile([65, N], F32, tag="hacc", name=f"phA{pr}"),
                       ps_h.tile([65, N], F32, tag="hacc", name=f"phB{pr}")]
                attnv = make_attnv(phs, pr)

                p_prev = [None, None]
                for j in range(8):
                    for side in range(2):
                        pb = 64 * side
                        sc = ps_sc.tile([128, N], F32, tag="sc")
                        for th in range(2):
                            nc.tensor.matmul(
                                sc[:, 512 * th:512 * (th + 1)],
                                qk[pb:pb + 64, k_base + 128 * j:k_base + 128 * (j + 1)],
                                qk[pb:pb + 64, q_base + 512 * th:q_base + 512 * (th + 1)],
                                start=True, stop=True,
                                tile_position=(pb, 0))
                        p_t = ppool.tile([128, N], BF16, tag="p")
                        nc.scalar.activation(p_t[:], sc[:], AF.Exp)
                        if p_prev[side] is not None:
                            attnv(side, j - 1, p_prev[side])
                        p_prev[side] = p_t
                        if j == 0 and side == 1 and tail is not None:
                            emit_tail(tail)
                            tail = None
                tail = (attnv, phs, p_prev, pr)
            emit_tail(tail)

        def emit_proj(st):
            hall4 = st.hall[:].rearrange("p (j i n) -> p j i n", j=2, i=2)
            for o in range(NT):
                pp = ps_h.tile([128, N], F32, tag="hacc", name=f"pp{o}")
                for kp in range(2):
                    for chk in CHK:
                        nc.tensor.matmul(
                            pp[:, 256 * chk:256 * (chk + 1)],
                            wp4[:, kp, :, 128 * o:128 * (o + 1)],
                            hall4[:, kp, :, 256 * chk:256 * (chk + 1)],
                            start=(kp == 0 and chk % 2 == 0),
                            stop=(kp == 1 and chk % 2 == 1), perf_mode=DR)
                for nh in range(2):
                    ot = opool.tile([128, 512], F32, tag="ot")
                    nc.vector.scalar_tensor_tensor(
                        ot[:], pp[:, 512 * nh:512 * (nh + 1)], 1.0 / (SW * HS),
                        st.xb[:, N * o + 512 * nh:N * o + 512 * (nh + 1)],
                        op0=OP.mult, op1=OP.add)
                    nc.sync.dma_start(
                        outd[st.b, 128 * o:128 * (o + 1), 512 * nh:512 * (nh + 1)],
                        ot[:])

        bs = [b for _ in range(reps) for b in range(BPC)]
        st = St()
        st.b = bs[0]
        emit_load(st)
        emit_stats(st)
        emit_gn_rest(st)
        emit_qkv(st)
        emit_v(st)
        for idx in range(len(bs)):
            if idx + 1 < len(bs):
                nxt = St()
                nxt.b = bs[idx + 1]
                emit_load(nxt)   # Pool DMA: transfers during heads(st)
                emit_stats(nxt)  # DVE-only: runs under heads(st)
            else:
                nxt = None
            emit_heads(st)
            if nxt is not None:
                # front-end of b+1 before proj(b): the PE covers proj's wait
                # on the last normalize, and heads(b+1) starts with qk ready
                emit_gn_rest(nxt)
                emit_qkv(nxt)
                emit_v(nxt)
            emit_proj(st)
            if nxt is not None:
                st = nxt

    nc.compile()
    return nc


def _prep_shared(w_qkv, b_qkv, w_proj, b_proj, gamma, beta):
    qs = 1.0 / np.sqrt(np.sqrt(float(CH)))  # ch**-0.25
    s2 = qs * qs
    r = np.arange(3 * C).reshape(NHEADS, 3, CH)
    idx_q, idx_k, idx_v = r[:, 0].ravel(), r[:, 1].ravel(), r[:, 2].ravel()
    wqk_t = np.ascontiguousarray(
        np.concatenate([w_qkv[idx_q], w_qkv[idx_k]], axis=0).T).astype(np.float32)
    wqk_t[:, :C] *= s2
    bqk_full = np.concatenate([b_qkv[idx_q], b_qkv[idx_k]])
    bqk_full[:C] *= s2
    bqk = np.ascontiguousarray(bqk_full.reshape(8, 128).T).astype(np.float32)

    wv = w_qkv[idx_v]
    bv_src = b_qkv[idx_v]
    wv_ext = np.zeros((C, VW), np.float32)
    bv_ext = np.zeros((VW,), np.float32)
    for h in range(NHEADS):
        wv_ext[:, (CH + 1) * h:(CH + 1) * h + CH] = wv[CH * h:CH * (h + 1), :].T
        bv_ext[(CH + 1) * h:(CH + 1) * h + CH] = bv_src[CH * h:CH * (h + 1)]
        bv_ext[(CH + 1) * h + CH] = 1.0
    bv_bc = np.ascontiguousarray(np.broadcast_to(bv_ext, (128, VW))).astype(np.float32)

    wp_t = np.ascontiguousarray(w_proj.T).astype(np.float32)
    bp = np.ascontiguousarray(b_proj.reshape(NT, 128).T).astype(np.float32)
    gamma_t = np.ascontiguousarray(gamma.reshape(NT, 128).T).astype(np.float32)
    beta_t = np.ascontiguousarray(beta.reshape(NT, 128).T).astype(np.float32)
    blockdiag16 = np.kron(np.eye(8, dtype=np.float32), np.ones((GSIZE, 1), np.float32))
    bcast16 = np.ascontiguousarray(blockdiag16.T)
    f8 = ml_dtypes.float8_e4m3
    rows = np.concatenate([idx_q, idx_k])
    wqk_raw = w_qkv[rows, :] * SW            # [1024 oc, 512 c], no s2
    wqk8 = np.ascontiguousarray(
        wqk_raw.T.reshape(4, 128, 1024).transpose(1, 0, 2).reshape(128, 4096)
    ).astype(f8)
    wv8 = np.ascontiguousarray(
        (wv_ext * SW).reshape(4, 128, VW).transpose(1, 0, 2).reshape(128, 4 * VW)
    ).astype(f8)
    wp8 = np.ascontiguousarray(
        (w_proj.T * SW).reshape(4, 128, C).transpose(1, 0, 2).reshape(128, 4 * C)
    ).astype(f8)
    return dict(wqk8=wqk8, bqk=bqk, wv8=wv8, bv_bc=bv_bc, wp8=wp8,
                bp=bp, gamma_t=gamma_t, beta_t=beta_t,
                blockdiag16=blockdiag16, bcast16=bcast16)


def kernel(x, gamma, beta, w_qkv, b_qkv, w_proj, b_proj):
    x = np.asarray(x, dtype=np.float32)
    shared = _prep_shared(np.asarray(w_qkv, np.float32), np.asarray(b_qkv, np.float32),
                          np.asarray(w_proj, np.float32), np.asarray(b_proj, np.float32),
                          np.asarray(gamma, np.float32), np.asarray(beta, np.float32))
    x6 = x.reshape(B, C, N)
    in_maps = [dict(x=np.ascontiguousarray(x6[BPC * i:BPC * (i + 1)]), **shared)
               for i in range(NCORES)]
    if "nc" not in _cached:
        _cached["nc"] = _build()
    res = run_bass_kernel_spmd(_cached["nc"], in_maps, list(range(NCORES)))
    out = np.empty((B, C, N), np.float32)
    for i in range(NCORES):
        out[BPC * i:BPC * (i + 1)] = res.results[i]["out"]
    return out.reshape(B, C, H, W)
